# revision 17
# baseline (speedup 1.0000x reference)
"""Trainium2 Bass kernel for nn_ExactScalarArray.

Math: the reference computes, per (b, l):  prod_k reduce(c1*c2, p1+p2)
in an exact ring representation Z[w], w = e^{i pi/4}, then converts to
complex and sums over l with power-of-two alignment.  The ring embed
into C is a homomorphism and the reduce step is value-preserving, so
the whole thing equals

    out[b] = sum_l ( prod_k v1(b,l,k) * v2(b,l,k) ) * 2^{S(b,l)}
    v(c)   = (c0 + (c1+c3)/sqrt2) + i (c2 + (c1-c3)/sqrt2)
    S      = sum_k (p1+p2)

evaluated here in f32 complex arithmetic (max rel err vs the reference
~9e-6, measured).  Sharding: batch dim B=256 split across 8 cores; all
reduction axes (K, L) are core-local, so no collectives.

Host-side, the four inputs are packed into one tensor per core so each
chunk is a single DMA (one wait on the first consumer; the TT ISA
struct only has one sync-wait slot).
"""

import numpy as np

import concourse.bass as bass
import concourse.mybir as mybir
import concourse.tile as tile
from concourse.bass_utils import run_bass_kernel_spmd

# Problem shape (hardcoded per contract)
B, L, K = 256, 2048, 8
NCORES = 8
BC = B // NCORES            # 32 batch rows per core
NR = BC * L                 # 65536 (b,l) rows per core
P = 128                     # SBUF partitions
RPP = NR // P               # 512 rows per partition
TC = 128                    # rows-per-partition per chunk
NCHUNK = RPP // TC          # 4 chunks
ROWW = 2 * K * 4 + 2 * K    # 80 packed f32 per row: c1(32) c2(32) p1(8) p2(8)
INV_SQRT2 = 0.7071067811865476

FP = mybir.dt.float32
I32 = mybir.dt.int32
ALU = mybir.AluOpType
AX = mybir.AxisListType


def build_program(split_waits=True):
    nc = bass.Bass("TRN2", target_bir_lowering=False, debug=False,
                   num_devices=NCORES)
    xind = nc.dram_tensor("xin", [P, RPP * ROWW], FP, kind="ExternalInput").ap()
    outd = nc.dram_tensor("out", [P, 2], FP, kind="ExternalOutput").ap()
    with tile.TileContext(nc) as tc:
        build_kernel(nc, tc, xind, outd)
    if split_waits:
        _split_multiwait(nc)
    return nc


def _split_multiwait(nc):
    """Walrus allows one sync-wait per ISA instruction; hoist extras onto
    NOPs inserted just before the offender on the same engine."""
    k = 0
    for f in nc.m.functions:
        for bb in f.blocks:
            il = bb.instructions
            i = 0
            while i < len(il):
                inst = il[i]
                si = inst.sync_info
                if si is not None and si.on_wait and len(si.on_wait) > 1:
                    waits = list(si.on_wait)
                    for w in waits[:-1]:
                        nop = mybir.InstNoOp(name=f"WSPLIT-{k}", ins=[], outs=[])
                        k += 1
                        nop.engine = inst.engine
                        nop.sync_info = mybir.SyncInfo(on_wait=[w], on_update=[])
                        il.insert(i, nop)
                        i += 1
                    si.on_wait = waits[-1:]
                    inst.sync_info = si
                i += 1


def build_kernel(nc, tc, xind, outd):
    FK = TC * K          # k-level free size per chunk (1024)
    FC = TC * K * 4      # coeff-level free size per chunk (4096)
    FX = TC * ROWW       # packed chunk free size (10240)

    with (
        tc.tile_pool(name="io", bufs=2) as io_pool,
        tc.tile_pool(name="head", bufs=4) as head_pool,
        tc.tile_pool(name="work", bufs=1) as work_pool,
        tc.tile_pool(name="acc", bufs=1) as acc_pool,
    ):
        acc_re = acc_pool.tile([P, NCHUNK], FP)
        acc_im = acc_pool.tile([P, NCHUNK], FP)

        xts = []     # per-chunk xt tile objects
        progs = []   # per-chunk "all xt reads done" markers (DVE-written)
        for ch in range(NCHUNK):
            xt = io_pool.tile([P, FX], FP, tag="xt")
            xts.append(xt)
            # Every ISA instruction has one sync-wait slot.  A reusing DMA
            # needs two waits (WAR vs the DVE readers of 2 chunks ago + WAW
            # vs that chunk's DMA); absorb each into its own tiny GPSIMD
            # fence copy issued ahead of the DMA on the same engine stream.
            fences = []
            if ch >= 2:
                fa = head_pool.tile([P, 1], FP, tag="fa")
                fb = head_pool.tile([P, 1], FP, tag="fb")
                fences.append(nc.gpsimd.tensor_copy(fa[:, :], progs[ch - 2][:, 0:1]))
                fences.append(nc.gpsimd.tensor_copy(fb[:, :], xts[ch - 2][:, 0:1]))
            dma = nc.gpsimd.dma_start(xt[:, :], xind[:, ch * FX:(ch + 1) * FX])
            for f in fences:
                tile.add_dep_helper(dma.ins, f.ins, False, "fence before dma")

            c1p = xt[:, 0:FC]
            c2p = xt[:, FC:2 * FC]
            p1p = xt[:, 2 * FC:2 * FC + FK]
            p2p = xt[:, 2 * FC + FK:2 * FC + 2 * FK]

            def comp(part, j):
                return part.rearrange("p (n c) -> p n c", c=4)[:, :, j]

            # t1 rotates through 4 buffers so the chunk's first consumer of
            # the DMA'd tile never also needs a same-engine WAR wait (the TT
            # ISA struct has a single sync-wait slot).
            t1 = head_pool.tile([P, FK], FP, tag="t1")
            t2 = work_pool.tile([P, FK], FP, tag="t2")
            t3 = work_pool.tile([P, FK], FP, tag="t3")
            t4 = work_pool.tile([P, FK], FP, tag="t4")
            re1 = work_pool.tile([P, FK], FP, tag="re1")
            im1 = work_pool.tile([P, FK], FP, tag="im1")
            re2 = work_pool.tile([P, FK], FP, tag="re2")
            im2 = work_pool.tile([P, FK], FP, tag="im2")

            # complexify: re = c0 + s*(c1+c3), im = c2 + s*(c1-c3)
            xt_readers = []
            xt_readers.append(nc.vector.tensor_tensor(
                t1[:, :], comp(c1p, 1), comp(c1p, 3), ALU.add))
            xt_readers.append(nc.vector.tensor_tensor(
                t2[:, :], comp(c1p, 1), comp(c1p, 3), ALU.subtract))
            xt_readers.append(nc.vector.tensor_tensor(
                t3[:, :], comp(c2p, 1), comp(c2p, 3), ALU.add))
            xt_readers.append(nc.vector.tensor_tensor(
                t4[:, :], comp(c2p, 1), comp(c2p, 3), ALU.subtract))
            xt_readers.append(nc.vector.scalar_tensor_tensor(
                re1[:, :], t1[:, :], INV_SQRT2, comp(c1p, 0), ALU.mult, ALU.add))
            xt_readers.append(nc.vector.scalar_tensor_tensor(
                im1[:, :], t2[:, :], INV_SQRT2, comp(c1p, 2), ALU.mult, ALU.add))
            xt_readers.append(nc.vector.scalar_tensor_tensor(
                re2[:, :], t3[:, :], INV_SQRT2, comp(c2p, 0), ALU.mult, ALU.add))
            xt_readers.append(nc.vector.scalar_tensor_tensor(
                im2[:, :], t4[:, :], INV_SQRT2, comp(c2p, 2), ALU.mult, ALU.add))

            # pairwise product w = v1*v2 (reuse t-tiles for partials)
            m1, m2, m3, m4 = t1, t2, t3, t4
            wre = work_pool.tile([P, FK], FP, tag="wre")
            wim = work_pool.tile([P, FK], FP, tag="wim")
            nc.vector.tensor_tensor(m1[:, :], re1[:, :], re2[:, :], ALU.mult)
            nc.vector.tensor_tensor(m2[:, :], im1[:, :], im2[:, :], ALU.mult)
            nc.vector.tensor_tensor(m3[:, :], re1[:, :], im2[:, :], ALU.mult)
            nc.vector.tensor_tensor(m4[:, :], im1[:, :], re2[:, :], ALU.mult)
            nc.vector.tensor_tensor(wre[:, :], m1[:, :], m2[:, :], ALU.subtract)
            nc.vector.tensor_tensor(wim[:, :], m3[:, :], m4[:, :], ALU.add)

            # product tree over K: 8 -> 4 -> 2 -> 1
            cur_re, cur_im = wre, wim
            width = FK
            lvl = 0
            while width > TC:
                width //= 2
                lvl += 1
                nre = work_pool.tile([P, width], FP, tag=f"lre{lvl}")
                nim = work_pool.tile([P, width], FP, tag=f"lim{lvl}")
                a_re = cur_re[:, 0:2 * width].rearrange("p (n two) -> p n two", two=2)
                a_im = cur_im[:, 0:2 * width].rearrange("p (n two) -> p n two", two=2)
                e_re, o_re = a_re[:, :, 0], a_re[:, :, 1]
                e_im, o_im = a_im[:, :, 0], a_im[:, :, 1]
                q1 = work_pool.tile([P, width], FP, tag=f"q1_{lvl}")
                q2 = work_pool.tile([P, width], FP, tag=f"q2_{lvl}")
                q3 = work_pool.tile([P, width], FP, tag=f"q3_{lvl}")
                q4 = work_pool.tile([P, width], FP, tag=f"q4_{lvl}")
                nc.vector.tensor_tensor(q1[:, :], e_re, o_re, ALU.mult)
                nc.vector.tensor_tensor(q2[:, :], e_im, o_im, ALU.mult)
                nc.vector.tensor_tensor(q3[:, :], e_re, o_im, ALU.mult)
                nc.vector.tensor_tensor(q4[:, :], e_im, o_re, ALU.mult)
                nc.vector.tensor_tensor(nre[:, :], q1[:, :], q2[:, :], ALU.subtract)
                nc.vector.tensor_tensor(nim[:, :], q3[:, :], q4[:, :], ALU.add)
                cur_re, cur_im = nre, nim

            # powers: S = sum_k (p1+p2); pw = 2^S exactly: (S+127)*2^23 is an
            # exact f32 integer, convert to i32, bitcast the bits back to f32.
            ps = work_pool.tile([P, FK], FP, tag="ps")
            S_t = work_pool.tile([P, TC], FP, tag="S_t")
            pq = work_pool.tile([P, TC], FP, tag="pq")
            pwi = work_pool.tile([P, TC], I32, tag="pwi")
            xt_readers.append(nc.vector.tensor_tensor(ps[:, :], p1p, p2p, ALU.add))
            prog = head_pool.tile([P, 1], FP, tag="prog")
            prog_copy = nc.vector.tensor_copy(prog[:, :], xt[:, 0:1])
            for r in xt_readers:
                tile.add_dep_helper(prog_copy.ins, r.ins, False, "prog after xt reads")
            progs.append(prog)
            nc.vector.tensor_reduce(
                S_t[:, :],
                ps[:, :].rearrange("p (n k) -> p n k", k=K),
                AX.X, ALU.add)
            nc.vector.tensor_scalar(
                pq[:, :], S_t[:, :], 127.0, float(1 << 23), ALU.add, ALU.mult)
            nc.vector.tensor_copy(pwi[:, :], pq[:, :])
            pw = pwi[:, :].bitcast(FP)

            # accumulate sum_l w * 2^S into this chunk's accumulator column
            dummy = work_pool.tile([P, TC], FP, tag="dummy")
            nc.vector.tensor_tensor(dummy[:, :], cur_re[:, :], pw, ALU.mult)
            nc.vector.tensor_reduce(acc_re[:, ch:ch + 1], dummy[:, :], AX.X, ALU.add)
            dummy2 = work_pool.tile([P, TC], FP, tag="dummy2")
            nc.vector.tensor_tensor(dummy2[:, :], cur_im[:, :], pw, ALU.mult)
            nc.vector.tensor_reduce(acc_im[:, ch:ch + 1], dummy2[:, :], AX.X, ALU.add)

        outt = acc_pool.tile([P, 2], FP)
        nc.vector.tensor_reduce(outt[:, 0:1], acc_re[:, :], AX.X, ALU.add)
        nc.vector.tensor_reduce(outt[:, 1:2], acc_im[:, :], AX.X, ALU.add)
        nc.gpsimd.dma_start(outd[:, :], outt[:, :])


_PROGRAM = None


def _get_program():
    global _PROGRAM
    if _PROGRAM is None:
        _PROGRAM = build_program()
    return _PROGRAM


def pack_core_input(c1, c2, p1, p2):
    """[BC,L,K,4]x2 f32 + [BC,L,K]x2 i32 -> packed [P, RPP*ROWW] f32.

    Rows (b*L+l) map to partition r//RPP, chunk (r%RPP)//TC; within a chunk
    the layout is [c1 block | c2 block | p1 block | p2 block]."""
    c1r = c1.reshape(P, NCHUNK, TC, K * 4)
    c2r = c2.reshape(P, NCHUNK, TC, K * 4)
    p1r = p1.astype(np.float32).reshape(P, NCHUNK, TC, K)
    p2r = p2.astype(np.float32).reshape(P, NCHUNK, TC, K)
    packed = np.empty((P, NCHUNK, TC * ROWW), dtype=np.float32)
    FC = TC * K * 4
    FK = TC * K
    packed[:, :, 0:FC] = c1r.reshape(P, NCHUNK, FC)
    packed[:, :, FC:2 * FC] = c2r.reshape(P, NCHUNK, FC)
    packed[:, :, 2 * FC:2 * FC + FK] = p1r.reshape(P, NCHUNK, FK)
    packed[:, :, 2 * FC + FK:] = p2r.reshape(P, NCHUNK, FK)
    return packed.reshape(P, RPP * ROWW)


def kernel(coeffs1, coeffs2, power1, power2):
    nc = _get_program()
    in_maps = []
    for ci in range(NCORES):
        sl = slice(ci * BC, (ci + 1) * BC)
        in_maps.append({
            "xin": pack_core_input(coeffs1[sl], coeffs2[sl],
                                   power1[sl], power2[sl]),
        })
    res = run_bass_kernel_spmd(nc, in_maps, core_ids=list(range(NCORES)))
    outs = []
    for ci in range(NCORES):
        o = res.results[ci]["out"]  # [128, 2]
        outs.append(o.reshape(BC, P // BC, 2).sum(axis=1, dtype=np.float32))
    return np.concatenate(outs, axis=0).astype(np.float32)


# revision 20
# speedup vs baseline: 1.3419x; 1.3419x over previous
"""Trainium2 Bass kernel for nn_ExactScalarArray.

Math: the reference computes, per (b, l):  prod_k reduce(c1*c2, p1+p2)
in an exact ring representation Z[w], w = e^{i pi/4}, then converts to
complex and sums over l with power-of-two alignment.  The ring embed
into C is a homomorphism and the reduce step is value-preserving, so
the whole thing equals

    out[b] = sum_l ( prod_k v1(b,l,k) * v2(b,l,k) ) * 2^{S(b,l)}
    v(c)   = (c0 + (c1+c3)/sqrt2) + i (c2 + (c1-c3)/sqrt2)
    S      = sum_k (p1+p2)

evaluated here in f32 complex arithmetic (max rel err vs the reference
~9e-6, measured).  Sharding: batch dim B=256 split across 8 cores; all
reduction axes (K, L) are core-local, so no collectives.

Host-side, the inputs (exact {0,1} values) are packed into dense bf16
component blocks inside one tensor per core: lossless, halves the HBM
traffic, and each chunk is a single DMA (one wait on the first
consumer; each ISA instruction has one sync-wait slot).
"""

import numpy as np

import concourse.bass as bass
import concourse.mybir as mybir
import concourse.tile as tile
from concourse.bass_utils import run_bass_kernel_spmd

# Problem shape (hardcoded per contract)
B, L, K = 256, 2048, 8
NCORES = 8
BC = B // NCORES            # 32 batch rows per core
NR = BC * L                 # 65536 (b,l) rows per core
P = 128                     # SBUF partitions
RPP = NR // P               # 512 rows per partition
TC = 128                    # rows-per-partition per chunk
NCHUNK = RPP // TC          # 4 chunks
NBLK = 10                   # bf16 blocks per chunk: c1_1,c1_3,c1_0,c1_2,
                            #   c2_1,c2_3,c2_0,c2_2, p1, p2  (each TC*K)
INV_SQRT2 = 0.7071067811865476

FP = mybir.dt.float32
BF = mybir.dt.bfloat16
I32 = mybir.dt.int32
ALU = mybir.AluOpType
AX = mybir.AxisListType

FK = TC * K              # one block, bf16 elements (1024)
FXF = TC * K * NBLK // 2  # packed chunk size in f32 units (5120)


def build_program(split_waits=True):
    nc = bass.Bass("TRN2", target_bir_lowering=False, debug=False,
                   num_devices=NCORES)
    xind = nc.dram_tensor("xin", [P, RPP * K * NBLK // 2], FP,
                          kind="ExternalInput").ap()
    outd = nc.dram_tensor("out", [P, 2], FP, kind="ExternalOutput").ap()
    with tile.TileContext(nc) as tc:
        build_kernel(nc, tc, xind, outd)
    if split_waits:
        _split_multiwait(nc)
    return nc


def _split_multiwait(nc):
    """Walrus allows one sync-wait per ISA instruction; hoist extras onto
    NOPs inserted just before the offender on the same engine."""
    k = 0
    for f in nc.m.functions:
        for bb in f.blocks:
            il = bb.instructions
            i = 0
            while i < len(il):
                inst = il[i]
                si = inst.sync_info
                if si is not None and si.on_wait and len(si.on_wait) > 1:
                    waits = list(si.on_wait)
                    for w in waits[:-1]:
                        nop = mybir.InstNoOp(name=f"WSPLIT-{k}", ins=[], outs=[])
                        k += 1
                        nop.engine = inst.engine
                        nop.sync_info = mybir.SyncInfo(on_wait=[w], on_update=[])
                        il.insert(i, nop)
                        i += 1
                    si.on_wait = waits[-1:]
                    inst.sync_info = si
                i += 1


def build_kernel(nc, tc, xind, outd):
    with (
        tc.tile_pool(name="io", bufs=2) as io_pool,
        tc.tile_pool(name="head", bufs=4) as head_pool,
        tc.tile_pool(name="work", bufs=1) as work_pool,
        tc.tile_pool(name="acc", bufs=1) as acc_pool,
    ):
        acc = acc_pool.tile([P, 2 * NCHUNK], FP)

        xts = []     # per-chunk xt tile objects
        progs = []   # per-chunk "all xt reads done" markers (DVE-written)
        for ch in range(NCHUNK):
            xt = io_pool.tile([P, FXF], FP, tag="xt")
            xts.append(xt)
            # Each ISA instruction has one sync-wait slot.  A reusing DMA
            # needs two waits (WAR vs the DVE readers of 2 chunks ago + WAW
            # vs that chunk's DMA); absorb each into its own tiny GPSIMD
            # fence copy issued ahead of the DMA on the same engine stream.
            fences = []
            if ch >= 2:
                fa = head_pool.tile([P, 1], FP, tag="fa")
                fb = head_pool.tile([P, 1], FP, tag="fb")
                fences.append(nc.gpsimd.tensor_copy(fa[:, :], progs[ch - 2][:, 0:1]))
                fences.append(nc.gpsimd.tensor_copy(fb[:, :], xts[ch - 2][:, 0:1]))
            dma = nc.gpsimd.dma_start(xt[:, :], xind[:, ch * FXF:(ch + 1) * FXF])
            for f in fences:
                tile.add_dep_helper(dma.ins, f.ins, False, "fence before dma")

            xb = xt[:, :].bitcast(BF)   # [P, NBLK*FK] bf16 blocks

            def blk(i, n=1):
                return xb[:, i * FK:(i + n) * FK]

            xt_readers = []

            # complexify both inputs.  t-tiles hold [t1|t2] = [c1+c3|c1-c3]
            # (exact small ints, bf16, 2x DVE mode); v-tiles hold [re|im] f32.
            # tj1 rotates through 4 buffers so the chunk's first consumer of
            # the DMA'd tile never also needs a same-engine WAR wait.
            tj1 = head_pool.tile([P, 2 * FK], BF, tag="tj1")
            tj2 = work_pool.tile([P, 2 * FK], BF, tag="tj2")
            v1 = work_pool.tile([P, 2 * FK], FP, tag="v1")
            v2 = work_pool.tile([P, 2 * FK], FP, tag="v2")
            xt_readers.append(nc.vector.tensor_tensor(
                tj1[:, 0:FK], blk(0), blk(1), ALU.add))
            xt_readers.append(nc.vector.tensor_tensor(
                tj1[:, FK:2 * FK], blk(0), blk(1), ALU.subtract))
            xt_readers.append(nc.vector.tensor_tensor(
                tj2[:, 0:FK], blk(4), blk(5), ALU.add))
            xt_readers.append(nc.vector.tensor_tensor(
                tj2[:, FK:2 * FK], blk(4), blk(5), ALU.subtract))
            xt_readers.append(nc.vector.scalar_tensor_tensor(
                v1[:, :], tj1[:, :], INV_SQRT2, blk(2, 2), ALU.mult, ALU.add))
            xt_readers.append(nc.vector.scalar_tensor_tensor(
                v2[:, :], tj2[:, :], INV_SQRT2, blk(6, 2), ALU.mult, ALU.add))

            # powers on GPSIMD (otherwise idle): S = sum_k (p1+p2), as a
            # strided add tree (GPSIMD tensor_reduce is partition-axis only)
            ps = work_pool.tile([P, FK], BF, tag="ps")
            xt_readers.append(nc.gpsimd.tensor_tensor(
                ps[:, :], blk(8), blk(9), ALU.add))
            pk = ps
            kwidth = FK
            while kwidth > TC:
                kwidth //= 2
                nk = work_pool.tile([P, kwidth], BF, tag=f"pk{kwidth}")
                nc.gpsimd.tensor_tensor(
                    nk[:, :], pk[:, 0:2 * kwidth:2], pk[:, 1:2 * kwidth:2],
                    ALU.add)
                pk = nk
            S_t = pk

            # pairwise product w = v1*v2, as [re|im] halves
            m12 = work_pool.tile([P, 2 * FK], FP, tag="m12")
            m34 = work_pool.tile([P, 2 * FK], FP, tag="m34")
            # v2 swapped halves: [im2|re2]
            v2sw = v2[:, :].rearrange("p (two n) -> p two n", two=2)[:, ::-1, :]
            nc.vector.tensor_tensor(m12[:, :], v1[:, :], v2[:, :], ALU.mult)
            nc.vector.tensor_tensor(m34[:, :], v1[:, :], v2sw, ALU.mult)
            w = work_pool.tile([P, 2 * FK], FP, tag="w0")
            nc.vector.tensor_tensor(
                w[:, 0:FK], m12[:, 0:FK], m12[:, FK:2 * FK], ALU.subtract)
            nc.vector.tensor_tensor(
                w[:, FK:2 * FK], m34[:, 0:FK], m34[:, FK:2 * FK], ALU.add)

            # product tree over K: 8 -> 4 -> 2 -> 1.  w layout [P, 2, width]
            width = FK
            lvl = 0
            while width > TC:
                width //= 2
                lvl += 1
                wv = w[:, :].rearrange("p (two n) -> p two n", two=2)
                ev = wv[:, :, 0::2]
                ov = wv[:, :, 1::2]
                ovsw = ov[:, ::-1, :]
                q12 = work_pool.tile([P, 2 * width], FP, tag=f"q12_{lvl}")
                q34 = work_pool.tile([P, 2 * width], FP, tag=f"q34_{lvl}")
                nc.vector.tensor_tensor(
                    q12[:, :].rearrange("p (two n) -> p two n", two=2),
                    ev, ov, ALU.mult)
                nc.vector.tensor_tensor(
                    q34[:, :].rearrange("p (two n) -> p two n", two=2),
                    ev, ovsw, ALU.mult)
                nw = work_pool.tile([P, 2 * width], FP, tag=f"w{lvl}")
                nc.vector.tensor_tensor(
                    nw[:, 0:width], q12[:, 0:width], q12[:, width:2 * width],
                    ALU.subtract)
                nc.vector.tensor_tensor(
                    nw[:, width:2 * width], q34[:, 0:width],
                    q34[:, width:2 * width], ALU.add)
                w = nw

            prog = head_pool.tile([P, 1], FP, tag="prog")
            prog_copy = nc.vector.tensor_copy(prog[:, :], xt[:, 0:1])
            for r in xt_readers:
                tile.add_dep_helper(prog_copy.ins, r.ins, False,
                                    "prog after xt reads")
            progs.append(prog)

            # pw = 2^S exactly: (S+127)*2^23 is an exact f32 integer; convert
            # to i32 and reinterpret the bits as f32.
            pq = work_pool.tile([P, TC], FP, tag="pq")
            pwi = work_pool.tile([P, TC], I32, tag="pwi")
            nc.vector.tensor_scalar(
                pq[:, :], S_t[:, :], 127.0, float(1 << 23), ALU.add, ALU.mult)
            nc.vector.tensor_copy(pwi[:, :], pq[:, :])
            pw = pwi[:, :].bitcast(FP)
            pwb = pw.rearrange("p (one n) -> p one n", one=1).broadcast_to(
                (P, 2, TC))

            # sum_l w * 2^S -> this chunk's [re, im] accumulator columns
            dummy = work_pool.tile([P, 2 * TC], FP, tag="dummy")
            nc.vector.tensor_tensor(
                dummy[:, :].rearrange("p (two n) -> p two n", two=2),
                w[:, :].rearrange("p (two n) -> p two n", two=2), pwb, ALU.mult)
            nc.vector.tensor_reduce(
                acc[:, 2 * ch:2 * ch + 2],
                dummy[:, :].rearrange("p (two n) -> p two n", two=2),
                AX.X, ALU.add)

        outt = acc_pool.tile([P, 2], FP)
        nc.vector.tensor_reduce(
            outt[:, :], acc[:, :].rearrange("p (c two) -> p two c", two=2),
            AX.X, ALU.add)
        nc.gpsimd.dma_start(outd[:, :], outt[:, :])


_PROGRAM = None


def _get_program():
    global _PROGRAM
    if _PROGRAM is None:
        _PROGRAM = build_program()
    return _PROGRAM


def _to_bf16_bits(a):
    """f32 array of exact small ints -> uint16 bf16 bit patterns."""
    return (np.ascontiguousarray(a, dtype=np.float32).view(np.uint32) >> 16
            ).astype(np.uint16)


def pack_core_input(c1, c2, p1, p2):
    """Pack one core's inputs into [P, RPP*K*NBLK/2] f32 (bf16 bit blocks).

    Rows (b*L+l) map to partition r//RPP, chunk (r%RPP)//TC; within a chunk
    there are NBLK dense bf16 blocks of TC*K values each:
    c1_1, c1_3, c1_0, c1_2, c2_1, c2_3, c2_0, c2_2, p1, p2."""
    u = np.empty((P, NCHUNK, NBLK, TC * K), dtype=np.uint16)

    def comp(c, j):
        return _to_bf16_bits(c[..., j]).reshape(P, NCHUNK, TC * K)

    u[:, :, 0] = comp(c1, 1)
    u[:, :, 1] = comp(c1, 3)
    u[:, :, 2] = comp(c1, 0)
    u[:, :, 3] = comp(c1, 2)
    u[:, :, 4] = comp(c2, 1)
    u[:, :, 5] = comp(c2, 3)
    u[:, :, 6] = comp(c2, 0)
    u[:, :, 7] = comp(c2, 2)
    u[:, :, 8] = _to_bf16_bits(p1.astype(np.float32)).reshape(P, NCHUNK, TC * K)
    u[:, :, 9] = _to_bf16_bits(p2.astype(np.float32)).reshape(P, NCHUNK, TC * K)
    return u.reshape(P, -1).view(np.float32)


def kernel(coeffs1, coeffs2, power1, power2):
    nc = _get_program()
    in_maps = []
    for ci in range(NCORES):
        sl = slice(ci * BC, (ci + 1) * BC)
        in_maps.append({
            "xin": pack_core_input(coeffs1[sl], coeffs2[sl],
                                   power1[sl], power2[sl]),
        })
    res = run_bass_kernel_spmd(nc, in_maps, core_ids=list(range(NCORES)))
    outs = []
    for ci in range(NCORES):
        o = res.results[ci]["out"]  # [128, 2]
        outs.append(o.reshape(BC, P // BC, 2).sum(axis=1, dtype=np.float32))
    return np.concatenate(outs, axis=0).astype(np.float32)
